# revision 31
# baseline (speedup 1.0000x reference)
"""AxialLinearAttention Trainium2 kernel (v5).

Data-parallel over batch across 8 NeuronCores (all math is batch-local).
Per core: feature-transposed activation layout (X^T: features on
partitions, tokens on the free dim); every projection is a dense
(128x128)@(128x512) bf16 matmul chain; linear attention is computed per
(head, 128-token chunk) as bf16 score matmuls with the axial group
structure applied as a constant block-diagonal mask fused into the PSUM
eviction.

v5 over v2 (the 1.23ms baseline):
 - x arrives pre-transposed (and pre-cast bf16) from the host as
   xT [D, T]; DMA (sync queue, parallel to the weight burst) lands
   tokens directly in the resident xt tiles -> no PE in-transposes,
   no transpose evictions, no staging tiles
 - output leaves feature-major as bf16 [D, T] straight from the
   residual tiles (host transposes back); no PE out-transposes, no
   ostage evictions. The residual is bf16 anyway, so bit-identical.
 - weights arrive bf16 from the host (half the prologue DMA traffic)
 - elu1(x) = min(exp(x), 1) + relu(x): exp on scalar, relu alternating
   scalar/vector, min/add combine on vector
 - v split into per-(feature-half, chunk) tiles so AV's dependency on
   each half is precise; evictions alternate scalar/vector engines
 - scores for the first two head-pairs are emitted between v-proj
   groups (keeps the DVE queue clear of v-copies at attention start);
   the rest pipeline LOOK=2 head-pairs ahead of AV consumption
 - supertiles are independent token sets: the next supertile's first
   PRE=3 ant q-proj groups are emitted inside the current supertile's
   freq out-proj, so the PE crosses supertile boundaries streaming
   dense independent matmuls (hides eviction tails + sem latency)
"""

import os
import sys

sys.path.insert(0, "/opt/trn_rl_repo")

import numpy as np

import concourse.bacc as bacc
import concourse.bass as bass
import concourse.mybir as mybir
import concourse.tile as tile

F32 = mybir.dt.float32
BF16 = mybir.dt.bfloat16
AF = mybir.ActivationFunctionType
ALU = mybir.AluOpType

B, FG, ANT, D = 256, 4, 32, 1024
H, DK = 16, 64
NCORES = 8
P = 128
NPT = D // P  # 8 feature partition-tiles

W_NAMES = [
    "ant_q_w", "ant_k_w", "ant_v_w", "ant_out_w",
    "freq_q_w", "freq_k_w", "freq_v_w", "freq_out_w",
]


def _emit_kernel(nc, tc, ctx, BC):
    T = BC * FG * ANT          # tokens per core
    ST = min(512, T)           # tokens per super-tile
    NST = T // ST
    SL = ST // 128             # 128-token chunks per super-tile
    HM = SL * 128              # scores width per head-parity

    x_d = nc.dram_tensor("xt", [D, T], BF16, kind="ExternalInput").ap()
    w_d = {n: nc.dram_tensor(n, [D, D], BF16, kind="ExternalInput").ap()
           for n in W_NAMES}
    out_d = nc.dram_tensor("out", [D, T], BF16, kind="ExternalOutput").ap()

    # ---- pools ----
    const_pool = ctx.enter_context(tc.tile_pool(name="const", bufs=1))
    wres = ctx.enter_context(tc.tile_pool(name="wres", bufs=1))
    big = ctx.enter_context(tc.tile_pool(name="big", bufs=1))
    xt_pool = ctx.enter_context(tc.tile_pool(name="xtp", bufs=2))
    tmp_p = ctx.enter_context(tc.tile_pool(name="tmp", bufs=2))
    sm_p = ctx.enter_context(tc.tile_pool(name="smp", bufs=12))
    ps_pj = ctx.enter_context(tc.tile_pool(name="ps_pj", bufs=4, space="PSUM"))
    ps_sc = ctx.enter_context(tc.tile_pool(name="ps_sc", bufs=3, space="PSUM"))
    ps_vv = ctx.enter_context(tc.tile_pool(name="ps_vv", bufs=1, space="PSUM"))

    # ---- constant score masks ----
    from concourse.masks import make_identity
    ident = const_pool.tile([P, P], F32)
    make_identity(nc, ident)
    # base masks, one 128-token chunk wide, replicated across HM columns.
    # ant: tokens grouped in contiguous blocks of 32 (groups = (b, fg))
    mant = const_pool.tile([P, P], F32)
    nc.gpsimd.memset(mant, 0.0)
    for g in range(4):
        nc.gpsimd.memset(mant[32 * g:32 * g + 32, 32 * g:32 * g + 32], 1.0)
    # freq: groups are (b, ant): l' interacts with l iff l' % 32 == l % 32
    mfrq = const_pool.tile([P, P], F32)
    nc.gpsimd.memset(mfrq, 0.0)
    for a in range(4):
        for b2 in range(4):
            nc.vector.tensor_copy(
                mfrq[32 * a:32 * a + 32, 32 * b2:32 * b2 + 32],
                ident[0:32, 0:32])
    mask_ant = const_pool.tile([P, HM], BF16)
    mask_freq = const_pool.tile([P, HM], BF16)
    for rep in range(HM // P):
        nc.vector.tensor_copy(mask_ant[:, rep * P:(rep + 1) * P], mant)
        nc.vector.tensor_copy(mask_freq[:, rep * P:(rep + 1) * P], mfrq)

    # ---- x tiles for supertile 0 (sync queue: runs in parallel with the
    # weight burst on the gpsimd queue) ----
    xts = {}

    def stage_x(s, slot):
        xt = []
        for i in range(NPT):
            t = xt_pool.tile([P, ST], BF16, tag=f"xt{i}", name=f"xt{i}_{slot}")
            nc.sync.dma_start(
                t, x_d[i * P:(i + 1) * P, s * ST:(s + 1) * ST])
            xt.append(t)
        xts[slot] = xt

    stage_x(0, 0)

    # ---- resident weights: two DMAs per matrix (half the i-tiles each),
    # emitted in first-supertile consumption order ----
    w_sb = {}
    for n in W_NAMES:
        w_sb[n] = wres.tile([P, NPT * D], BF16, tag=n, name=n)
    for n in W_NAMES:
        w = w_sb[n]
        for ih in range(2):
            h = NPT // 2
            nc.gpsimd.dma_start(
                w[:, ih * h * D:(ih + 1) * h * D].rearrange(
                    "p (i j) -> p i j", i=h),
                w_d[n][ih * h * P:(ih + 1) * h * P, :].rearrange(
                    "(i p) j -> p i j", p=P))

    def wsl(n, i, j0, j1):
        """lhsT block: rows = feature tile i, cols j0:j1 of W[n]."""
        return w_sb[n][:, i * D + j0:i * D + j1]

    # ================= per-super-tile emission =================

    def proj_T_groups(wn, src, elu, dst_tag):
        """out^T[j] = sum_i W[i,j]^T @ src^T[i], one emitter per j-PAIR.

        The two groups of a pair are emitted round-robin over i so
        consecutive PE matmuls hit different PSUM accumulation groups:
        same-group back-to-back accumulates pay a ~128-cycle pipeline
        bubble on hardware (probe-measured +53ns/matmul), interleaving
        hides it under the sibling group's stream.
        """
        dst = [None] * NPT

        def evict(j, ps):
            o = big.tile([P, ST], BF16, tag=f"{dst_tag}{j}",
                         name=f"{dst_tag}{j}")
            if elu:
                # elu1(x) = min(exp(x), 1) + relu(x); relu alternates
                # engines so neither scalar nor vector saturates
                e = tmp_p.tile([P, ST], BF16, tag="e")
                nc.scalar.activation(e, ps, AF.Exp)
                rl = tmp_p.tile([P, ST], BF16, tag="r")
                if j % 2 == 0:
                    nc.vector.tensor_scalar_max(rl, ps, 0.0)
                else:
                    nc.scalar.activation(rl, ps, AF.Relu)
                nc.vector.scalar_tensor_tensor(
                    o, e, 1.0, rl, op0=ALU.min, op1=ALU.add)
            else:
                nc.scalar.activation(o, ps, AF.Copy)
            dst[j] = o

        def emit_pair(jp):
            ja, jb = jp, jp + 1
            psa = ps_pj.tile([P, ST], F32, tag="pj")
            psb = ps_pj.tile([P, ST], F32, tag="pj")
            for i in range(NPT):
                nc.tensor.matmul(
                    psa, lhsT=wsl(wn, i, ja * P, (ja + 1) * P), rhs=src[i],
                    start=(i == 0), stop=(i == NPT - 1))
                nc.tensor.matmul(
                    psb, lhsT=wsl(wn, i, jb * P, (jb + 1) * P), rhs=src[i],
                    start=(i == 0), stop=(i == NPT - 1))
            evict(ja, psa)
            evict(jb, psb)

        return dst, emit_pair

    def proj_T(wn, src, elu, dst_tag):
        dst, emit_pair = proj_T_groups(wn, src, elu, dst_tag)
        for jp in range(0, NPT, 2):
            emit_pair(jp)
        return dst

    def proj_V(wn, src, interleave=None):
        """X^T-stationary projection -> V in natural (token, feature) layout.

        v[j2][sl] holds feature half j2 (hp 0-3 / 4-7) of token chunk sl as
        its own tile so AV's dependency on each half is precise; evictions
        alternate scalar/vector so the tail lands sooner.
        """
        v = [[big.tile([P, 512], BF16, tag=f"v{j2}{sl}", name=f"v{j2}{sl}")
              for sl in range(SL)] for j2 in range(2)]
        pending = list(interleave or [])
        groups = [(j2, sl) for j2 in range(2) for sl in range(SL)]
        for ga, gb in zip(groups[0::2], groups[1::2]):
            # round-robin the pair over i: consecutive matmuls hit
            # different PSUM groups (hides the accumulate bubble)
            psa = ps_vv.tile([P, HM], F32, tag="vv")
            psb = ps_pj.tile([P, HM], F32, tag="pj")
            for i in range(NPT):
                for (j2, sl), ps in ((ga, psa), (gb, psb)):
                    nc.tensor.matmul(
                        ps,
                        lhsT=src[i][:, sl * P:(sl + 1) * P],
                        rhs=wsl(wn, i, j2 * 512, (j2 + 1) * 512),
                        start=(i == 0), stop=(i == NPT - 1))
            for (j2, sl), ps in ((ga, psa), (gb, psb)):
                if (sl + j2) % 2 == 0:
                    nc.scalar.activation(v[j2][sl], ps, AF.Copy)
                else:
                    nc.vector.tensor_copy(v[j2][sl], ps)
            if pending:
                pending.pop(0)()
        for t in pending:
            t()
        return v

    def make_scores(qt, kt, mask, sms):
        """Emit the masked score computation for one (head-pair, parity)."""

        def emit(hp, par):
            off = 64 * par
            sp = ps_sc.tile([P, HM], F32, tag="sc")
            for c in range(SL):
                nc.tensor.matmul(
                    sp[:, c * P:(c + 1) * P],
                    lhsT=kt[hp][off:off + 64, c * P:(c + 1) * P],
                    rhs=qt[hp][off:off + 64, c * P:(c + 1) * P],
                    start=True, stop=True)
            sm = sm_p.tile([P, HM], BF16, tag="sm")
            nc.vector.tensor_tensor(sm, sp, mask, op=ALU.mult)
            sms.setdefault(hp, []).append(sm)

        return emit

    def attention(qt, kt, v, sms, scores, pre):
        """AV per head-pair; scores for hp >= pre are emitted LOOK ahead."""
        at = []
        LOOK = int(os.environ.get("K_LOOK", "2"))
        for hp in range(pre, min(pre + LOOK, NPT)):
            scores(hp, 0)
            scores(hp, 1)
        for hp in range(NPT):
            nxt = hp + LOOK
            if pre + LOOK <= nxt < NPT:
                scores(nxt, 0)
                scores(nxt, 1)
            ap_ = ps_pj.tile([P, ST], F32, tag="pj")
            vh = v[hp // 4]
            vo = (hp % 4) * P
            for par in range(2):
                off = 64 * par
                for c in range(SL):
                    nc.tensor.matmul(
                        ap_[off:off + 64, c * P:(c + 1) * P],
                        lhsT=vh[c][:, vo + off:vo + off + 64],
                        rhs=sms[hp][par][:, c * P:(c + 1) * P],
                        start=True, stop=True)
            del sms[hp]
            o = big.tile([P, ST], BF16, tag=f"at{hp}", name=f"at{hp}")
            if hp % 2 == 0:
                nc.vector.tensor_copy(o, ap_)
            else:
                nc.scalar.activation(o, ap_, AF.Copy)
            at.append(o)
        return at

    def outproj_residual(wn, at, res, writeout=None, interleave=None):
        """res^T[j] += W_o[:,j]^T @ A^T  (in-place residual update).

        writeout: supertile index -> after each res[j] update, DMA it out.
        interleave: thunks run one per group (next supertile's q-proj
        emissions) so the PE has independent work at the boundary.
        """
        pending = list(interleave or [])
        for jp in range(0, NPT, 2):
            psa = ps_pj.tile([P, ST], F32, tag="pj")
            psb = ps_pj.tile([P, ST], F32, tag="pj")
            for i in range(NPT):
                nc.tensor.matmul(
                    psa, lhsT=wsl(wn, i, jp * P, (jp + 1) * P), rhs=at[i],
                    start=(i == 0), stop=(i == NPT - 1))
                nc.tensor.matmul(
                    psb, lhsT=wsl(wn, i, (jp + 1) * P, (jp + 2) * P),
                    rhs=at[i], start=(i == 0), stop=(i == NPT - 1))
            for j, ps in ((jp, psa), (jp + 1, psb)):
                nc.vector.tensor_add(res[j], ps, res[j])
                if writeout is not None:
                    s = writeout
                    nc.sync.dma_start(
                        out_d[j * P:(j + 1) * P, s * ST:(s + 1) * ST], res[j])
            if pending:
                t = pending.pop(0)
                if t is not None:
                    t()
        for t in pending:
            if t is not None:
                t()

    # ================= main loop =================
    # Supertiles are independent token sets: the next supertile's ant q-proj
    # groups are pre-emitted into the current supertile's freq out-proj so the
    # PE crosses the boundary with dense independent matmul work (PRE groups).
    REP = int(os.environ.get("K_REPEAT", "1"))  # timing experiments only
    NOATT = os.environ.get("K_PROBE", "") == "noattn"
    PREP = int(os.environ.get("K_PREQ", "2"))  # q-proj PAIRS pre-emitted
    NTOT = NST * REP
    next_q = None  # (dst, emit_pair) for next supertile's ant q-proj
    for s_ in range(NTOT):
        s = s_ % NST
        slot = s_ % 2
        xt = xts[slot]
        # prefetch next supertile's tokens into the other buffer slot
        if s_ + 1 < NTOT:
            stage_x((s_ + 1) % NST, 1 - slot)
        for blk, mask in (("ant", mask_ant), ("freq", mask_freq)):
            if blk == "ant" and next_q is not None:
                qdst, qemit = next_q
                for jp in range(2 * PREP, NPT, 2):
                    qemit(jp)
                qt = qdst
            else:
                qt = proj_T(f"{blk}_q_w", xt, True, "qt")
            kt = proj_T(f"{blk}_k_w", xt, True, "kt")
            if NOATT:
                v = proj_V(f"{blk}_v_w", xt)
                at = qt  # timing probe: skip attention, outproj reads qt
            else:
                sms = {}
                scores = make_scores(qt, kt, mask, sms)
                SPRE = 2
                thunks = [(lambda hp=hp, par=par: scores(hp, par))
                          for hp in range(SPRE) for par in range(2)]
                v = proj_V(f"{blk}_v_w", xt, interleave=thunks)
                at = attention(qt, kt, v, sms, scores, SPRE)
            del v
            thunks = None
            if blk == "freq" and s_ + 1 < NTOT:
                next_q = proj_T_groups("ant_q_w", xts[1 - slot], True, "qt")
                _, qemit = next_q
                thunks = ([None] * (NPT // 2 - PREP)
                          + [(lambda jp=2 * p: qemit(jp))
                             for p in range(PREP)])
            outproj_residual(
                f"{blk}_out_w", at, xt,
                writeout=(s if blk == "freq" else None),
                interleave=thunks)


def build(BC):
    from contextlib import ExitStack

    nc = bacc.Bacc("TRN2", target_bir_lowering=False, debug=False)
    with tile.TileContext(nc) as tc:
        with ExitStack() as ctx:
            _emit_kernel(nc, tc, ctx, BC)
    nc.compile()
    return nc


_CACHE = {}
last_results = None


def _prep_core_inputs(x, weights_bf16, k, BC):
    import ml_dtypes
    xk = x[k * BC:(k + 1) * BC].reshape(BC * FG * ANT, D)
    m = {"xt": np.ascontiguousarray(xk.T).astype(ml_dtypes.bfloat16)}
    m.update(weights_bf16)
    return m


def kernel(x, **inputs):
    """Full (unsharded) inputs -> full output. Shards batch across 8 cores."""
    global last_results
    import ml_dtypes
    from concourse.bass_utils import run_bass_kernel_spmd

    x = np.ascontiguousarray(np.asarray(x), dtype=np.float32)
    BC = B // NCORES
    if "nc" not in _CACHE:
        _CACHE["nc"] = build(BC)
    nc = _CACHE["nc"]

    weights = {n: np.asarray(inputs[n]).astype(ml_dtypes.bfloat16)
               for n in W_NAMES}
    in_maps = [_prep_core_inputs(x, weights, k, BC) for k in range(NCORES)]

    res = run_bass_kernel_spmd(nc, in_maps, core_ids=list(range(NCORES)))
    last_results = res
    out = np.empty((B, FG * ANT, D), dtype=np.float32)
    for k in range(NCORES):
        out[k * BC:(k + 1) * BC] = (
            res.results[k]["out"].astype(np.float32).T
            .reshape(BC, FG * ANT, D))
    return out


# revision 36
# speedup vs baseline: 1.0242x; 1.0242x over previous
"""AxialLinearAttention Trainium2 kernel (v5).

Data-parallel over batch across 8 NeuronCores (all math is batch-local).
Per core: feature-transposed activation layout (X^T: features on
partitions, tokens on the free dim); every projection is a dense
(128x128)@(128x512) bf16 matmul chain; linear attention is computed per
(head, 128-token chunk) as bf16 score matmuls with the axial group
structure applied as a constant block-diagonal mask fused into the PSUM
eviction.

v5 over v2 (the 1.23ms baseline):
 - x arrives pre-transposed (and pre-cast bf16) from the host as
   xT [D, T]; DMA (sync queue, parallel to the weight burst) lands
   tokens directly in the resident xt tiles -> no PE in-transposes,
   no transpose evictions, no staging tiles
 - output leaves feature-major as bf16 [D, T] straight from the
   residual tiles (host transposes back); no PE out-transposes, no
   ostage evictions. The residual is bf16 anyway, so bit-identical.
 - weights arrive bf16 from the host (half the prologue DMA traffic)
 - elu1(x) = min(exp(x), 1) + relu(x): exp on scalar, relu alternating
   scalar/vector, min/add combine on vector
 - v split into per-(feature-half, chunk) tiles so AV's dependency on
   each half is precise; evictions alternate scalar/vector engines
 - scores for the first two head-pairs are emitted between v-proj
   groups (keeps the DVE queue clear of v-copies at attention start);
   the rest pipeline LOOK=2 head-pairs ahead of AV consumption
 - supertiles are independent token sets: the next supertile's first
   PRE=3 ant q-proj groups are emitted inside the current supertile's
   freq out-proj, so the PE crosses supertile boundaries streaming
   dense independent matmuls (hides eviction tails + sem latency)
"""

import os
import sys

sys.path.insert(0, "/opt/trn_rl_repo")

import numpy as np

import concourse.bacc as bacc
import concourse.bass as bass
import concourse.mybir as mybir
import concourse.tile as tile

F32 = mybir.dt.float32
BF16 = mybir.dt.bfloat16
AF = mybir.ActivationFunctionType
ALU = mybir.AluOpType

B, FG, ANT, D = 256, 4, 32, 1024
H, DK = 16, 64
NCORES = 8
P = 128
NPT = D // P  # 8 feature partition-tiles

W_NAMES = [
    "ant_q_w", "ant_k_w", "ant_v_w", "ant_out_w",
    "freq_q_w", "freq_k_w", "freq_v_w", "freq_out_w",
]


def _emit_kernel(nc, tc, ctx, BC):
    T = BC * FG * ANT          # tokens per core
    ST = min(512, T)           # tokens per super-tile
    NST = T // ST
    SL = ST // 128             # 128-token chunks per super-tile
    HM = SL * 128              # scores width per head-parity

    x_d = nc.dram_tensor("xt", [D, T], BF16, kind="ExternalInput").ap()
    w_d = {n: nc.dram_tensor(n, [D, D], BF16, kind="ExternalInput").ap()
           for n in W_NAMES}
    out_d = nc.dram_tensor("out", [D, T], BF16, kind="ExternalOutput").ap()

    # ---- pools ----
    const_pool = ctx.enter_context(tc.tile_pool(name="const", bufs=1))
    wres = ctx.enter_context(tc.tile_pool(name="wres", bufs=1))
    big = ctx.enter_context(tc.tile_pool(name="big", bufs=1))
    xt_pool = ctx.enter_context(tc.tile_pool(name="xtp", bufs=2))
    tmp_p = ctx.enter_context(tc.tile_pool(name="tmp", bufs=2))
    sm_p = ctx.enter_context(tc.tile_pool(name="smp", bufs=12))
    ps_pj = ctx.enter_context(tc.tile_pool(name="ps_pj", bufs=4, space="PSUM"))
    ps_sc = ctx.enter_context(tc.tile_pool(name="ps_sc", bufs=3, space="PSUM"))
    ps_vv = ctx.enter_context(tc.tile_pool(name="ps_vv", bufs=1, space="PSUM"))

    # ---- constant score masks ----
    from concourse.masks import make_identity
    ident = const_pool.tile([P, P], F32)
    make_identity(nc, ident)
    # base masks, one 128-token chunk wide, replicated across HM columns.
    # ant: tokens grouped in contiguous blocks of 32 (groups = (b, fg))
    mant = const_pool.tile([P, P], F32)
    nc.gpsimd.memset(mant, 0.0)
    for g in range(4):
        nc.gpsimd.memset(mant[32 * g:32 * g + 32, 32 * g:32 * g + 32], 1.0)
    # freq: groups are (b, ant): l' interacts with l iff l' % 32 == l % 32
    mfrq = const_pool.tile([P, P], F32)
    nc.gpsimd.memset(mfrq, 0.0)
    for a in range(4):
        for b2 in range(4):
            nc.vector.tensor_copy(
                mfrq[32 * a:32 * a + 32, 32 * b2:32 * b2 + 32],
                ident[0:32, 0:32])
    mask_ant = const_pool.tile([P, HM], BF16)
    mask_freq = const_pool.tile([P, HM], BF16)
    for rep in range(HM // P):
        nc.vector.tensor_copy(mask_ant[:, rep * P:(rep + 1) * P], mant)
        nc.vector.tensor_copy(mask_freq[:, rep * P:(rep + 1) * P], mfrq)

    # ---- x tiles for supertile 0 (sync queue: runs in parallel with the
    # weight burst on the gpsimd queue) ----
    xts = {}

    def stage_x(s, slot):
        xt = []
        for i in range(NPT):
            t = xt_pool.tile([P, ST], BF16, tag=f"xt{i}", name=f"xt{i}_{slot}")
            nc.sync.dma_start(
                t, x_d[i * P:(i + 1) * P, s * ST:(s + 1) * ST])
            xt.append(t)
        xts[slot] = xt

    stage_x(0, 0)

    # ---- resident weights: two DMAs per matrix (half the i-tiles each),
    # emitted in first-supertile consumption order ----
    w_sb = {}
    for n in W_NAMES:
        w_sb[n] = wres.tile([P, NPT * D], BF16, tag=n, name=n)
    for n in W_NAMES:
        w = w_sb[n]
        for ih in range(2):
            h = NPT // 2
            nc.gpsimd.dma_start(
                w[:, ih * h * D:(ih + 1) * h * D].rearrange(
                    "p (i j) -> p i j", i=h),
                w_d[n][ih * h * P:(ih + 1) * h * P, :].rearrange(
                    "(i p) j -> p i j", p=P))

    def wsl(n, i, j0, j1):
        """lhsT block: rows = feature tile i, cols j0:j1 of W[n]."""
        return w_sb[n][:, i * D + j0:i * D + j1]

    # ================= per-super-tile emission =================

    def proj_T_groups(wn, src, elu, dst_tag):
        """out^T[j] = sum_i W[i,j]^T @ src^T[i], one emitter per j-group."""
        dst = [None] * NPT

        def emit(j):
            ps = ps_pj.tile([P, ST], F32, tag="pj")
            for i in range(NPT):
                nc.tensor.matmul(
                    ps, lhsT=wsl(wn, i, j * P, (j + 1) * P), rhs=src[i],
                    start=(i == 0), stop=(i == NPT - 1))
            o = big.tile([P, ST], BF16, tag=f"{dst_tag}{j}",
                         name=f"{dst_tag}{j}")
            if elu:
                # elu1(x) = min(exp(x), 1) + relu(x); relu alternates
                # engines so neither scalar nor vector saturates
                e = tmp_p.tile([P, ST], BF16, tag="e")
                nc.scalar.activation(e, ps, AF.Exp)
                rl = tmp_p.tile([P, ST], BF16, tag="r")
                if j % 2 == 0:
                    nc.vector.tensor_scalar_max(rl, ps, 0.0)
                else:
                    nc.scalar.activation(rl, ps, AF.Relu)
                nc.vector.scalar_tensor_tensor(
                    o, e, 1.0, rl, op0=ALU.min, op1=ALU.add)
            else:
                nc.scalar.activation(o, ps, AF.Copy)
            dst[j] = o

        return dst, emit

    def proj_T(wn, src, elu, dst_tag, skip=0):
        dst, emit = proj_T_groups(wn, src, elu, dst_tag)
        for j in range(skip, NPT):
            emit(j)
        return dst

    def proj_V(wn, src, interleave=None):
        """X^T-stationary projection -> V in natural (token, feature) layout.

        v[j2][sl] holds feature half j2 (hp 0-3 / 4-7) of token chunk sl as
        its own tile so AV's dependency on each half is precise; evictions
        alternate scalar/vector so the tail lands sooner.
        """
        v = [[big.tile([P, 512], BF16, tag=f"v{j2}{sl}", name=f"v{j2}{sl}")
              for sl in range(SL)] for j2 in range(2)]
        pending = list(interleave or [])
        for j2 in range(2):
            for sl in range(SL):
                ps = ps_vv.tile([P, HM], F32, tag="vv")
                for i in range(NPT):
                    nc.tensor.matmul(
                        ps,
                        lhsT=src[i][:, sl * P:(sl + 1) * P],
                        rhs=wsl(wn, i, j2 * 512, (j2 + 1) * 512),
                        start=(i == 0), stop=(i == NPT - 1))
                if sl % 2 == 0:
                    nc.scalar.activation(v[j2][sl], ps, AF.Copy)
                else:
                    nc.vector.tensor_copy(v[j2][sl], ps)
                if pending:
                    pending.pop(0)()
        for t in pending:
            t()
        return v

    def make_scores(qt, kt, mask, sms):
        """Emit the masked score computation for one (head-pair, parity)."""

        def emit(hp, par):
            off = 64 * par
            sp = ps_sc.tile([P, HM], F32, tag="sc")
            for c in range(SL):
                nc.tensor.matmul(
                    sp[:, c * P:(c + 1) * P],
                    lhsT=kt[hp][off:off + 64, c * P:(c + 1) * P],
                    rhs=qt[hp][off:off + 64, c * P:(c + 1) * P],
                    start=True, stop=True)
            sm = sm_p.tile([P, HM], BF16, tag="sm")
            nc.vector.tensor_tensor(sm, sp, mask, op=ALU.mult)
            sms.setdefault(hp, []).append(sm)

        return emit

    def attention(qt, kt, v, sms, scores, pre):
        """AV per head-pair; scores for hp >= pre are emitted LOOK ahead."""
        at = []
        LOOK = 2
        for hp in range(pre, min(pre + LOOK, NPT)):
            scores(hp, 0)
            scores(hp, 1)
        for hp in range(NPT):
            nxt = hp + LOOK
            if pre + LOOK <= nxt < NPT:
                scores(nxt, 0)
                scores(nxt, 1)
            ap_ = ps_pj.tile([P, ST], F32, tag="pj")
            vh = v[hp // 4]
            vo = (hp % 4) * P
            for par in range(2):
                off = 64 * par
                for c in range(SL):
                    nc.tensor.matmul(
                        ap_[off:off + 64, c * P:(c + 1) * P],
                        lhsT=vh[c][:, vo + off:vo + off + 64],
                        rhs=sms[hp][par][:, c * P:(c + 1) * P],
                        start=True, stop=True)
            del sms[hp]
            o = big.tile([P, ST], BF16, tag=f"at{hp}", name=f"at{hp}")
            if hp % 2 == 0:
                nc.vector.tensor_copy(o, ap_)
            else:
                nc.scalar.activation(o, ap_, AF.Copy)
            at.append(o)
        return at

    def outproj_residual(wn, at, res, writeout=None, interleave=None):
        """res^T[j] += W_o[:,j]^T @ A^T  (in-place residual update).

        writeout: supertile index -> after each res[j] update, DMA it out.
        interleave: thunks run one per group (next supertile's q-proj
        emissions) so the PE has independent work at the boundary.
        """
        pending = list(interleave or [])
        for j in range(NPT):
            ps = ps_pj.tile([P, ST], F32, tag="pj")
            for i in range(NPT):
                nc.tensor.matmul(
                    ps, lhsT=wsl(wn, i, j * P, (j + 1) * P), rhs=at[i],
                    start=(i == 0), stop=(i == NPT - 1))
            nc.vector.tensor_add(res[j], ps, res[j])
            if writeout is not None:
                s = writeout
                nc.sync.dma_start(
                    out_d[j * P:(j + 1) * P, s * ST:(s + 1) * ST], res[j])
            if pending:
                t = pending.pop(0)
                if t is not None:
                    t()
        for t in pending:
            if t is not None:
                t()

    # ================= main loop =================
    # Supertiles are independent token sets: the next supertile's ant q-proj
    # groups are pre-emitted into the current supertile's freq out-proj so the
    # PE crosses the boundary with dense independent matmul work (PRE groups).
    REP = int(os.environ.get("K_REPEAT", "1"))  # timing experiments only
    NOATT = os.environ.get("K_PROBE", "") == "noattn"
    PRE = int(os.environ.get("K_PREQ", "3"))
    NTOT = NST * REP
    next_q = None  # (dst, emit) for supertile s_'s ant q-proj, PRE pre-emitted
    for s_ in range(NTOT):
        s = s_ % NST
        slot = s_ % 2
        xt = xts[slot]
        # prefetch next supertile's tokens into the other buffer slot
        if s_ + 1 < NTOT:
            stage_x((s_ + 1) % NST, 1 - slot)
        for blk, mask in (("ant", mask_ant), ("freq", mask_freq)):
            if blk == "ant" and next_q is not None:
                qdst, qemit = next_q
                for j in range(PRE, NPT):
                    qemit(j)
                qt = qdst
            else:
                qt = proj_T(f"{blk}_q_w", xt, True, "qt")
            kt = proj_T(f"{blk}_k_w", xt, True, "kt")
            if NOATT:
                v = proj_V(f"{blk}_v_w", xt)
                at = qt  # timing probe: skip attention, outproj reads qt
            else:
                sms = {}
                scores = make_scores(qt, kt, mask, sms)
                SPRE = 2
                thunks = [(lambda hp=hp, par=par: scores(hp, par))
                          for hp in range(SPRE) for par in range(2)]
                v = proj_V(f"{blk}_v_w", xt, interleave=thunks)
                at = attention(qt, kt, v, sms, scores, SPRE)
            del v
            thunks = None
            if blk == "freq" and s_ + 1 < NTOT:
                next_q = proj_T_groups("ant_q_w", xts[1 - slot], True, "qt")
                _, qemit = next_q
                thunks = ([None] * (NPT - PRE)
                          + [(lambda j=j: qemit(j)) for j in range(PRE)])
            outproj_residual(
                f"{blk}_out_w", at, xt,
                writeout=(s if blk == "freq" else None),
                interleave=thunks)


def build(BC):
    from contextlib import ExitStack

    nc = bacc.Bacc("TRN2", target_bir_lowering=False, debug=False)
    with tile.TileContext(nc) as tc:
        with ExitStack() as ctx:
            _emit_kernel(nc, tc, ctx, BC)
    nc.compile()
    return nc


_CACHE = {}
last_results = None


def _prep_core_inputs(x, weights_bf16, k, BC):
    import ml_dtypes
    xk = x[k * BC:(k + 1) * BC].reshape(BC * FG * ANT, D)
    m = {"xt": np.ascontiguousarray(xk.T).astype(ml_dtypes.bfloat16)}
    m.update(weights_bf16)
    return m


def kernel(x, **inputs):
    """Full (unsharded) inputs -> full output. Shards batch across 8 cores."""
    global last_results
    import ml_dtypes
    from concourse.bass_utils import run_bass_kernel_spmd

    x = np.ascontiguousarray(np.asarray(x), dtype=np.float32)
    BC = B // NCORES
    if "nc" not in _CACHE:
        _CACHE["nc"] = build(BC)
    nc = _CACHE["nc"]

    weights = {n: np.asarray(inputs[n]).astype(ml_dtypes.bfloat16)
               for n in W_NAMES}
    in_maps = [_prep_core_inputs(x, weights, k, BC) for k in range(NCORES)]

    res = run_bass_kernel_spmd(nc, in_maps, core_ids=list(range(NCORES)))
    last_results = res
    out = np.empty((B, FG * ANT, D), dtype=np.float32)
    for k in range(NCORES):
        out[k * BC:(k + 1) * BC] = (
            res.results[k]["out"].astype(np.float32).T
            .reshape(BC, FG * ANT, D))
    return out
